# revision 1
# baseline (speedup 1.0000x reference)
"""Causal self-attention (RMS-normed QK + partial RoPE + lambda-blended V)
for Trainium2, tensor-parallel over heads across 8 NeuronCores.

Per core: 2 heads. Device pipeline per 512-token block:
  x -> bf16 -> DRAM scratch -> DMA-transpose -> xT
  QKV matmuls (bf16, fp32 psum) -> RMS (ACT square+accum, exp(-.5 ln)) -> RoPE (DVE)
  -> PE-transpose q,k -> flash-style causal attention (scores^T, ACT exp,
  mask, fp32 l accumulation, PV accumulation in psum) -> per-head 1/l fold
  into output projection -> partial out [T, D] fp32.
Host: shards weights per core, sums the 8 partial outputs.
"""
import sys
sys.path.insert(0, "/opt/trn_rl_repo")

import math
import numpy as np
import ml_dtypes

import concourse.bass as bass
import concourse.tile as tile
from concourse import bacc, mybir
from concourse.masks import make_identity

bf16 = ml_dtypes.bfloat16
F32 = mybir.dt.float32
BF = mybir.dt.bfloat16
AF = mybir.ActivationFunctionType
ALU = mybir.AluOpType

D = 2048          # model dim
NH = 16           # total heads
DH = 128          # head dim
NCORES = 8
HPC = NH // NCORES          # heads per core = 2
DLOC = HPC * DH             # local hdim = 256
EPS = 1e-6
TB = 512                    # t-block size
SQRT_DH = math.sqrt(DH)

_BUILD_CACHE = {}


def _build(T):
    """Build the per-core Bass program (same program on all cores)."""
    NTB = T // TB
    nc = bacc.Bacc("TRN2", target_bir_lowering=False)

    x_in = nc.dram_tensor("x", [T, D], F32, kind="ExternalInput")
    wq_in = nc.dram_tensor("wqkv", [D, 3 * DLOC], BF, kind="ExternalInput")
    wp_in = nc.dram_tensor("wproj", [DLOC, D], BF, kind="ExternalInput")
    ve_in = nc.dram_tensor("ve", [T, DLOC], BF, kind="ExternalInput")
    cos_in = nc.dram_tensor("cos", [T, 32], F32, kind="ExternalInput")
    sin_in = nc.dram_tensor("sin", [T, 32], F32, kind="ExternalInput")
    mask_in = nc.dram_tensor("mask", [128, 4, TB], BF, kind="ExternalInput")
    out_d = nc.dram_tensor("out", [T, D], F32, kind="ExternalOutput")

    with tile.TileContext(nc) as tc:
        with (
            tc.tile_pool(name="const", bufs=1) as const,
            tc.tile_pool(name="res", bufs=1) as res,
            tc.tile_pool(name="xa", bufs=2) as xa,
            tc.tile_pool(name="xb", bufs=2) as xb,
            tc.tile_pool(name="xt", bufs=1) as xtp,
            tc.tile_pool(name="work", bufs=2) as work,
            tc.tile_pool(name="att", bufs=3) as att,
            tc.tile_pool(name="accp", bufs=2) as accp,
            tc.tile_pool(name="prj", bufs=2) as prj,
            tc.tile_pool(name="psA", bufs=2, space="PSUM") as psA,
            tc.tile_pool(name="psB", bufs=1, space="PSUM") as psB,
            tc.tile_pool(name="psC", bufs=1, space="PSUM") as psC,
            tc.tile_pool(name="psD", bufs=2, space="PSUM") as psD,
            tc.tile_pool(name="dram", bufs=1, space="DRAM") as dram,
        ):
            # ---------------- constants ----------------
            wq_sb = const.tile([128, D // 128, 3 * DLOC], BF, tag="wq")
            nc.sync.dma_start(wq_sb[:], wq_in.rearrange("(c p) e -> p c e", p=128))
            wp_sb = const.tile([128, HPC, D], BF, tag="wp")
            nc.sync.dma_start(wp_sb[:], wp_in.rearrange("(h p) e -> p h e", p=128))
            cos_sb = const.tile([128, T // 128, 32], F32, tag="cos")
            nc.sync.dma_start(cos_sb[:], cos_in.rearrange("(c p) f -> p c f", p=128))
            sin_sb = const.tile([128, T // 128, 32], F32, tag="sin")
            nc.sync.dma_start(sin_sb[:], sin_in.rearrange("(c p) f -> p c f", p=128))
            mask_sb = const.tile([128, 4, TB], BF, tag="mask")
            nc.sync.dma_start(mask_sb[:], mask_in[:])
            ident = const.tile([128, 128], BF, tag="ident")
            make_identity(nc, ident[:])
            ones = const.tile([128, 1], F32, tag="ones")
            nc.vector.memset(ones[:], 1.0)
            lnbias = const.tile([128, 1], F32, tag="lnbias")
            nc.vector.memset(lnbias[:], float(EPS * SQRT_DH))

            # ---------------- resident per-block tensors ----------------
            qT = [res.tile([128, HPC, TB], BF, tag=f"qT{i}", name=f"qT{i}") for i in range(NTB)]
            kT = [res.tile([128, HPC, TB], BF, tag=f"kT{i}", name=f"kT{i}") for i in range(NTB)]
            vB = [res.tile([128, 4, DLOC], BF, tag=f"v{i}", name=f"v{i}") for i in range(NTB)]
            xdr = [dram.tile([TB, D], BF, tag=f"xdr{i}", name=f"xdr{i}") for i in range(NTB)]

            for ti in range(NTB):
                t0 = ti * TB
                # ============ QKV stage ============
                for sub in range(4):
                    x_nat = xa.tile([128, D], F32, tag="xnat")
                    nc.sync.dma_start(x_nat[:], x_in[t0 + sub * 128: t0 + (sub + 1) * 128, :])
                    x_bfs = xb.tile([128, D], BF, tag="xbf")
                    nc.gpsimd.tensor_copy(x_bfs[:], x_nat[:])
                    nc.sync.dma_start(xdr[ti][sub * 128:(sub + 1) * 128, :], x_bfs[:])
                xt = xtp.tile([128, D // 128, TB], BF, tag="xt")
                for dc in range(D // 128):
                    nc.sync.dma_start_transpose(xt[:, dc, :], xdr[ti][:, dc * 128:(dc + 1) * 128])

                for sub in range(4):
                    tg = ti * 4 + sub
                    qkv_ps = psA.tile([128, 1024], F32, tag="big")
                    ndc = D // 128
                    for dc in range(ndc):
                        lhsT = xt[:, dc, sub * 128:(sub + 1) * 128]
                        st, sp = dc == 0, dc == ndc - 1
                        # q|k share psum bank 0 as one N=512 group; v is bank 1
                        nc.tensor.matmul(qkv_ps[:, 0:512], lhsT, wq_sb[:, dc, 0:512], start=st, stop=sp)
                        nc.tensor.matmul(qkv_ps[:, 512:768], lhsT, wq_sb[:, dc, 512:768], start=st, stop=sp)
                    # v (lambda0 pre-folded in weights; lambda1*ve DMA-accumulated below)
                    nc.any.tensor_copy(vB[ti][:, sub, :], qkv_ps[:, 512:768])
                    # rms statistics: sumsq per head for q and k
                    ssq = work.tile([128, 4], F32, tag="ssq")
                    sq_scr = work.tile([128, 512], BF, tag="sqscr")
                    for i in range(4):
                        nc.scalar.activation(
                            sq_scr[:, i * 128:(i + 1) * 128], qkv_ps[:, i * 128:(i + 1) * 128],
                            AF.Square, accum_out=ssq[:, i:i + 1])
                    # rstd' = (sqrt(DH)*(ms+eps))^-1/2 = exp(-0.5*ln(...)); folds the
                    # 1/sqrt(DH) score scale (split as DH^-0.25 into q and k each)
                    rstd = work.tile([128, 4], F32, tag="rstd")
                    nc.scalar.activation(rstd[:], ssq[:], AF.Ln,
                                         scale=float(SQRT_DH / DH), bias=lnbias[:])
                    nc.scalar.activation(rstd[:], rstd[:], AF.Exp, scale=-0.5)
                    # normalize + cast
                    qn = work.tile([128, HPC, DH], BF, tag="qn")
                    kn = work.tile([128, HPC, DH], BF, tag="kn")
                    for h in range(HPC):
                        nc.any.tensor_scalar_mul(qn[:, h, :], qkv_ps[:, h * 128:(h + 1) * 128], rstd[:, h:h + 1])
                        nc.any.tensor_scalar_mul(kn[:, h, :], qkv_ps[:, 256 + h * 128:256 + (h + 1) * 128], rstd[:, 2 + h:3 + h])
                    # rope (first 32 freq pairs only; rest are identity)
                    cosb = cos_sb[:, tg, :][:, None, :].broadcast_to([128, HPC, 32])
                    sinb = sin_sb[:, tg, :][:, None, :].broadcast_to([128, HPC, 32])
                    for tl in (qn, kn):
                        x1 = tl[:, :, 0:32]
                        x2 = tl[:, :, 64:96]
                        r1 = work.tile([128, HPC, 32], BF, tag="r1")
                        r2 = work.tile([128, HPC, 32], BF, tag="r2")
                        r3 = work.tile([128, HPC, 32], BF, tag="r3")
                        r4 = work.tile([128, HPC, 32], BF, tag="r4")
                        nc.vector.tensor_mul(r1[:], x1, cosb)
                        nc.vector.tensor_mul(r2[:], x2, sinb)
                        nc.vector.tensor_mul(r3[:], x1, sinb)
                        nc.vector.tensor_mul(r4[:], x2, cosb)
                        nc.vector.tensor_add(x1, r1[:], r2[:])
                        nc.vector.tensor_sub(x2, r4[:], r3[:])
                    # transpose q,k into resident [d, t] layout
                    for h in range(HPC):
                        for tl, dstl in ((qn, qT), (kn, kT)):
                            tp = psC.tile([128, 128], BF, tag="tp")
                            nc.tensor.transpose(tp[:], tl[:, h, :], ident[:])
                            nc.any.tensor_copy(dstl[ti][:, h, sub * 128:(sub + 1) * 128], tp[:])
                # blend ve into v via accumulating DMA
                nc.gpsimd.dma_start(
                    vB[ti][:], ve_in[t0:t0 + TB, :].rearrange("(c p) d -> p c d", p=128),
                    accum_op=ALU.add)

                # ============ attention stage ============
                linv = prj.tile([128, HPC, 4], F32, tag="linv")
                oB = prj.tile([128, HPC, TB], BF, tag="o")
                ns = (ti + 1) * 4
                for h in range(HPC):
                    l_acc = accp.tile([128, TB], F32, tag="lacc")
                    nc.vector.memset(l_acc[:], 0.0)
                    o_ps = psB.tile([128, TB], F32, tag="o")
                    for sj2 in range(0, ns, 2):
                        sc_ps = psA.tile([128, 1024], F32, tag="big")
                        for k2 in range(2):
                            sj = sj2 + k2
                            blk, sb_ = sj // 4, sj % 4
                            nc.tensor.matmul(
                                sc_ps[:, k2 * 512:(k2 + 1) * 512],
                                kT[blk][:, h, sb_ * 128:(sb_ + 1) * 128],
                                qT[ti][:, h, :], start=True, stop=True)
                        probs = att.tile([128, 1024], BF, tag="probs")
                        nc.scalar.activation(probs[:], sc_ps[:], AF.Exp)
                        for k2 in range(2):
                            j = sj2 + k2 - ti * 4
                            if j >= 0:  # diagonal block: causal mask
                                nc.vector.tensor_mul(
                                    probs[:, k2 * 512:(k2 + 1) * 512],
                                    probs[:, k2 * 512:(k2 + 1) * 512], mask_sb[:, j, :])
                        nc.vector.tensor_add(l_acc[:], l_acc[:], probs[:, 0:512])
                        nc.vector.tensor_add(l_acc[:], l_acc[:], probs[:, 512:1024])
                        for k2 in range(2):
                            sj = sj2 + k2
                            blk, sb_ = sj // 4, sj % 4
                            nc.tensor.matmul(
                                o_ps[:], vB[blk][:, sb_, h * 128:(h + 1) * 128],
                                probs[:, k2 * 512:(k2 + 1) * 512],
                                start=(sj == 0), stop=(sj == ns - 1))
                    # transposed partition-reduce of l (fp32 matmul, N=1)
                    lcol = psC.tile([128, 4], F32, tag="tp")
                    for c in range(4):
                        nc.tensor.matmul(lcol[:, c:c + 1], l_acc[:, c * 128:(c + 1) * 128],
                                         ones[:], start=(c == 0), stop=(c == 3))
                    nc.vector.reciprocal(linv[:, h, :], lcol[:])
                    nc.any.tensor_copy(oB[:, h, :], o_ps[:])

                # ============ projection stage ============
                for sub in range(4):
                    out_sb = prj.tile([128, D], F32, tag="outsb")
                    for dn in range(D // 512):
                        pr0 = psD.tile([128, 512], F32, tag="pr")
                        nc.tensor.matmul(pr0[:], oB[:, 0, sub * 128:(sub + 1) * 128],
                                         wp_sb[:, 0, dn * 512:(dn + 1) * 512], start=True, stop=True)
                        tmp = prj.tile([128, 512], F32, tag="tmp")
                        nc.any.tensor_scalar_mul(tmp[:], pr0[:], linv[:, 0, sub:sub + 1])
                        pr1 = psD.tile([128, 512], F32, tag="pr")
                        nc.tensor.matmul(pr1[:], oB[:, 1, sub * 128:(sub + 1) * 128],
                                         wp_sb[:, 1, dn * 512:(dn + 1) * 512], start=True, stop=True)
                        nc.vector.scalar_tensor_tensor(
                            out_sb[:, dn * 512:(dn + 1) * 512], pr1[:], linv[:, 1, sub:sub + 1],
                            tmp[:], op0=ALU.mult, op1=ALU.add)
                    nc.sync.dma_start(out_d[t0 + sub * 128: t0 + (sub + 1) * 128, :], out_sb[:])
    return nc


def _host_prep(x, ve, lambdas, qkv_w, proj_w, T):
    """Build the 8 per-core input maps (sharding + constant tables)."""
    x = np.ascontiguousarray(np.asarray(x, np.float32).reshape(T, D))
    ve = np.asarray(ve, np.float32).reshape(T, NH * DH)
    lam = np.asarray(lambdas, np.float32)
    qkv_w = np.asarray(qkv_w, np.float32)
    proj_w = np.asarray(proj_w, np.float32)

    quarter = DH // 4
    ang = (1.0 / 1024.0) ** np.linspace(0.0, 1.0, quarter, dtype=np.float32)
    theta = np.arange(T, dtype=np.float32)[:, None] * ang[None, :]   # [T, 32]
    cos_t = np.cos(theta).astype(np.float32)
    sin_t = np.sin(theta).astype(np.float32)

    s_l = np.arange(128)[:, None]
    t_l = np.arange(TB)[None, :]
    mask = np.stack([(t_l >= s_l + 128 * j) for j in range(4)], axis=1).astype(bf16)  # [128,4,TB]

    in_maps = []
    for c in range(NCORES):
        sl = slice(c * DLOC, (c + 1) * DLOC)
        wqkv = np.concatenate(
            [qkv_w[0, sl].T, qkv_w[1, sl].T, lam[0] * qkv_w[2, sl].T], axis=1)  # [D, 768]
        in_maps.append({
            "x": x,
            "wqkv": np.ascontiguousarray(wqkv).astype(bf16),
            "wproj": np.ascontiguousarray(proj_w[:, sl].T).astype(bf16),
            "ve": np.ascontiguousarray(lam[1] * ve[:, sl]).astype(bf16),
            "cos": cos_t, "sin": sin_t, "mask": mask,
        })
    return in_maps


def kernel(x, ve, lambdas, qkv_w, proj_w):
    B, T, _ = x.shape
    in_maps = _host_prep(x, ve, lambdas, qkv_w, proj_w, T)
    if T not in _BUILD_CACHE:
        nc = _build(T)
        nc.compile()
        _BUILD_CACHE[T] = nc
    nc = _BUILD_CACHE[T]

    from concourse.bass_utils import run_bass_kernel_spmd
    res = run_bass_kernel_spmd(nc, in_maps, core_ids=list(range(NCORES)))
    out = np.zeros((T, D), np.float32)
    for c in range(NCORES):
        out += res.results[c]["out"]
    return out.reshape(B, T, D)



# revision 3
# speedup vs baseline: 1.2052x; 1.2052x over previous
"""Causal self-attention (RMS-normed QK + partial RoPE + lambda-blended V)
for Trainium2, tensor-parallel over heads across 8 NeuronCores.

Per core: 2 heads. Host pre-tiles all inputs (x transposed to [d, t] bf16
tiles) so every DMA is 128 contiguous per-partition descriptors and the
device does no casts / transposes of x.

Device pipeline per 512-token block:
  xt tile DMA -> QKV matmuls (x-tile stationary, bf16, fp32 psum)
  -> RMS (ACT square+accum, exp(-.5 ln)) -> RoPE (DVE, q+k fused tiles)
  -> PE-transpose q,k into resident [d, t] tiles
  -> flash-style causal attention (scores^T per 128-key sub-block, ACT exp,
     diagonal mask, bf16 l accumulation, PV accumulation in psum)
  -> l partition-reduced via ones-matmul (fp32 psum), reciprocal,
     PE outer-product broadcast of 1/l, fold into o
  -> out projection with both heads accumulated in one psum bank,
     ACT evacuation -> partial out [T, D] fp32.
Host: shards weights per core, sums the 8 partial outputs.
"""
import sys
sys.path.insert(0, "/opt/trn_rl_repo")

import math
import numpy as np
import ml_dtypes

import concourse.bass as bass
import concourse.tile as tile
from concourse import bacc, mybir
from concourse.masks import make_identity

bf16 = ml_dtypes.bfloat16
F32 = mybir.dt.float32
BF = mybir.dt.bfloat16
AF = mybir.ActivationFunctionType
ALU = mybir.AluOpType

D = 2048          # model dim
NH = 16           # total heads
DH = 128          # head dim
NCORES = 8
HPC = NH // NCORES          # heads per core = 2
DLOC = HPC * DH             # local hdim = 256
EPS = 1e-6
TB = 512                    # t-block size
SQRT_DH = math.sqrt(DH)

_BUILD_CACHE = {}


def _build(T):
    """Build the per-core Bass program (same program on all cores)."""
    NTB = T // TB
    NC128 = T // 128
    nc = bacc.Bacc("TRN2", target_bir_lowering=False)

    xt_in = nc.dram_tensor("xt", [NTB, 128, D // 128, TB], BF, kind="ExternalInput")
    ve_in = nc.dram_tensor("ve", [NTB, 128, 4, DLOC], BF, kind="ExternalInput")
    wq_in = nc.dram_tensor("wqkv", [128, D // 128, 3 * DLOC], BF, kind="ExternalInput")
    wp_in = nc.dram_tensor("wproj", [128, HPC, D], BF, kind="ExternalInput")
    cos_in = nc.dram_tensor("cos", [128, NC128, 32], F32, kind="ExternalInput")
    sin_in = nc.dram_tensor("sin", [128, NC128, 32], F32, kind="ExternalInput")
    mask_in = nc.dram_tensor("mask", [128, 4, TB], BF, kind="ExternalInput")
    out_d = nc.dram_tensor("out", [T, D], F32, kind="ExternalOutput")

    with tile.TileContext(nc) as tc:
        with (
            tc.tile_pool(name="const", bufs=1) as const,
            tc.tile_pool(name="res", bufs=1) as res,
            tc.tile_pool(name="xt", bufs=2) as xtp,
            tc.tile_pool(name="vep", bufs=2) as vep,
            tc.tile_pool(name="work", bufs=2) as work,
            tc.tile_pool(name="att", bufs=3) as att,
            tc.tile_pool(name="lac", bufs=2) as lac,
            tc.tile_pool(name="lrw", bufs=2) as lrw,
            tc.tile_pool(name="lbc", bufs=2) as lbcp,
            tc.tile_pool(name="ob", bufs=2) as obp,
            tc.tile_pool(name="prj", bufs=2) as prj,
            tc.tile_pool(name="psA", bufs=2, space="PSUM") as psA,
            tc.tile_pool(name="psB", bufs=1, space="PSUM") as psB,
            tc.tile_pool(name="psC", bufs=1, space="PSUM") as psC,
            tc.tile_pool(name="psD", bufs=2, space="PSUM") as psD,
        ):
            # ---------------- constants ----------------
            wq_sb = const.tile([128, D // 128, 3 * DLOC], BF, tag="wq")
            nc.sync.dma_start(wq_sb[:], wq_in[:])
            wp_sb = const.tile([128, HPC, D], BF, tag="wp")
            nc.sync.dma_start(wp_sb[:], wp_in[:])
            cos_sb = const.tile([128, NC128, 32], F32, tag="cos")
            nc.sync.dma_start(cos_sb[:], cos_in[:])
            sin_sb = const.tile([128, NC128, 32], F32, tag="sin")
            nc.sync.dma_start(sin_sb[:], sin_in[:])
            mask_sb = const.tile([128, 4, TB], BF, tag="mask")
            nc.sync.dma_start(mask_sb[:], mask_in[:])
            ident = const.tile([128, 128], BF, tag="ident")
            make_identity(nc, ident[:])
            ones_bf = const.tile([128, 1], BF, tag="onesb")
            nc.vector.memset(ones_bf[:], 1.0)
            ones1_f = const.tile([1, 128], F32, tag="ones1")
            nc.vector.memset(ones1_f[:], 1.0)
            lnbias = const.tile([128, 1], F32, tag="lnbias")
            nc.vector.memset(lnbias[:], float(EPS * SQRT_DH))

            # ---------------- resident per-block tensors ----------------
            # qkT[i]: [d, (q0,q1,k0,k1), t] transposed q/k
            qkT = [res.tile([128, 4, TB], BF, tag=f"qkT{i}", name=f"qkT{i}")
                   for i in range(NTB)]
            vB = [res.tile([128, 4, DLOC], BF, tag=f"v{i}", name=f"v{i}")
                  for i in range(NTB)]

            for ti in range(NTB):
                # ============ QKV stage ============
                xt = xtp.tile([128, D // 128, TB], BF, tag="xt")
                nc.sync.dma_start(xt[:], xt_in[ti])
                ve_sb = vep.tile([128, 4, DLOC], BF, tag="ve")
                nc.sync.dma_start(ve_sb[:], ve_in[ti])

                for sub in range(4):
                    tg = ti * 4 + sub
                    qkv_ps = psA.tile([128, 1024], F32, tag="big")
                    ndc = D // 128
                    for dc in range(ndc):
                        lhsT = xt[:, dc, sub * 128:(sub + 1) * 128]
                        st, sp = dc == 0, dc == ndc - 1
                        # q0|q1|k0|k1 share psum bank 0; v is bank 1
                        nc.tensor.matmul(qkv_ps[:, 0:512], lhsT, wq_sb[:, dc, 0:512], start=st, stop=sp)
                        nc.tensor.matmul(qkv_ps[:, 512:768], lhsT, wq_sb[:, dc, 512:768], start=st, stop=sp)
                    # v blend: vB = v_psum + lam1*ve (lam0 folded in weights)
                    nc.vector.tensor_add(vB[ti][:, sub, :], qkv_ps[:, 512:768], ve_sb[:, sub, :])
                    # rms statistics: sumsq per head for q and k
                    ssq = work.tile([128, 4], F32, tag="ssq")
                    sq_scr = work.tile([128, 512], BF, tag="sqscr")
                    for i in range(4):
                        nc.scalar.activation(
                            sq_scr[:, i * 128:(i + 1) * 128], qkv_ps[:, i * 128:(i + 1) * 128],
                            AF.Square, accum_out=ssq[:, i:i + 1])
                    # rstd' = (sqrt(DH)*(ms+eps))^-1/2; folds the 1/sqrt(DH)
                    # score scale (split as DH^-0.25 into q and k each)
                    rstd = work.tile([128, 4], F32, tag="rstd")
                    nc.scalar.activation(rstd[:], ssq[:], AF.Ln,
                                         scale=float(SQRT_DH / DH), bias=lnbias[:])
                    nc.scalar.activation(rstd[:], rstd[:], AF.Exp, scale=-0.5)
                    # normalize + cast, q0 q1 k0 k1 into one fused tile
                    qkn = work.tile([128, 4, DH], BF, tag="qkn")
                    for i in range(4):
                        nc.any.tensor_scalar_mul(qkn[:, i, :], qkv_ps[:, i * 128:(i + 1) * 128], rstd[:, i:i + 1])
                    # rope (first 32 freq pairs only; rest identity), q&k fused
                    cosb = cos_sb[:, tg, :][:, None, :].broadcast_to([128, 4, 32])
                    sinb = sin_sb[:, tg, :][:, None, :].broadcast_to([128, 4, 32])
                    x1 = qkn[:, :, 0:32]
                    x2 = qkn[:, :, 64:96]
                    r1 = work.tile([128, 4, 32], BF, tag="r1")
                    r2 = work.tile([128, 4, 32], BF, tag="r2")
                    r3 = work.tile([128, 4, 32], BF, tag="r3")
                    r4 = work.tile([128, 4, 32], BF, tag="r4")
                    nc.vector.tensor_mul(r1[:], x1, cosb)
                    nc.vector.tensor_mul(r2[:], x2, sinb)
                    nc.vector.tensor_mul(r3[:], x1, sinb)
                    nc.vector.tensor_mul(r4[:], x2, cosb)
                    nc.vector.tensor_add(x1, r1[:], r2[:])
                    nc.vector.tensor_sub(x2, r4[:], r3[:])
                    # transpose q,k into resident [d, t] layout; batch the
                    # 4 results into one psum tile -> single copy out
                    tp4 = psC.tile([128, 4, 128], BF, tag="tp4")
                    for i in range(4):
                        nc.tensor.transpose(tp4[:, i, :], qkn[:, i, :], ident[:])
                    nc.any.tensor_copy(qkT[ti][:, :, sub * 128:(sub + 1) * 128], tp4[:])

                # ============ attention stage ============
                oB = obp.tile([128, HPC, TB], BF, tag="o")
                ns = (ti + 1) * 4
                for h in range(HPC):
                    l_acc = lac.tile([128, 1024], BF, tag="lacc")
                    nc.vector.memset(l_acc[:], 0.0)
                    o_ps = psB.tile([128, TB], F32, tag="o")
                    for sj2 in range(0, ns, 2):
                        sc_ps = psA.tile([128, 1024], F32, tag="big")
                        for k2 in range(2):
                            sj = sj2 + k2
                            blk, sb_ = sj // 4, sj % 4
                            nc.tensor.matmul(
                                sc_ps[:, k2 * 512:(k2 + 1) * 512],
                                qkT[blk][:, 2 + h, sb_ * 128:(sb_ + 1) * 128],
                                qkT[ti][:, h, :], start=True, stop=True)
                        probs = att.tile([128, 1024], BF, tag="probs")
                        nc.scalar.activation(probs[:], sc_ps[:], AF.Exp)
                        for k2 in range(2):
                            j = sj2 + k2 - ti * 4
                            if j >= 0:  # diagonal block: causal mask
                                nc.vector.tensor_mul(
                                    probs[:, k2 * 512:(k2 + 1) * 512],
                                    probs[:, k2 * 512:(k2 + 1) * 512], mask_sb[:, j, :])
                        # l accumulation (bf16; partition-reduce later is fp32)
                        nc.vector.tensor_add(l_acc[:, 0:512], l_acc[:, 0:512], probs[:, 0:512])
                        nc.vector.tensor_add(l_acc[:, 512:1024], l_acc[:, 512:1024], probs[:, 512:1024])
                        for k2 in range(2):
                            sj = sj2 + k2
                            blk, sb_ = sj // 4, sj % 4
                            nc.tensor.matmul(
                                o_ps[:], vB[blk][:, sb_, h * 128:(h + 1) * 128],
                                probs[:, k2 * 512:(k2 + 1) * 512],
                                start=(sj == 0), stop=(sj == ns - 1))
                    # l[t] = sum_s l_acc[s, t] via ones-matmul (fp32 in psum)
                    lr = psD.tile([1, TB], F32, tag="pr")
                    nc.tensor.matmul(lr[:], ones_bf[:], l_acc[:, 0:512], start=True, stop=False)
                    nc.tensor.matmul(lr[:], ones_bf[:], l_acc[:, 512:1024], start=False, stop=True)
                    linv_row = lrw.tile([1, TB], F32, tag="linv")
                    nc.vector.reciprocal(linv_row[:], lr[:])
                    # broadcast 1/l across partitions via PE outer product
                    lbc_ps = psD.tile([128, TB], F32, tag="pr")
                    nc.tensor.matmul(lbc_ps[:], ones1_f[:], linv_row[:], start=True, stop=True)
                    lbc = lbcp.tile([128, TB], BF, tag="lbc")
                    nc.scalar.copy(lbc[:], lbc_ps[:])
                    # fold 1/l into o while evacuating psum
                    nc.vector.tensor_mul(oB[:, h, :], o_ps[:], lbc[:])

                # ============ projection stage ============
                t0 = ti * TB
                for sub in range(4):
                    out_sb = prj.tile([128, D], F32, tag="outsb")
                    for dn in range(D // 512):
                        pr = psD.tile([128, 512], F32, tag="pr")
                        nc.tensor.matmul(pr[:], oB[:, 0, sub * 128:(sub + 1) * 128],
                                         wp_sb[:, 0, dn * 512:(dn + 1) * 512], start=True, stop=False)
                        nc.tensor.matmul(pr[:], oB[:, 1, sub * 128:(sub + 1) * 128],
                                         wp_sb[:, 1, dn * 512:(dn + 1) * 512], start=False, stop=True)
                        if dn % 2 == 0:
                            nc.scalar.copy(out_sb[:, dn * 512:(dn + 1) * 512], pr[:])
                        else:
                            nc.vector.tensor_copy(out_sb[:, dn * 512:(dn + 1) * 512], pr[:])
                    nc.sync.dma_start(out_d[t0 + sub * 128: t0 + (sub + 1) * 128, :], out_sb[:])
    return nc


def _host_prep(x, ve, lambdas, qkv_w, proj_w, T):
    """Build the 8 per-core input maps (sharding + pre-tiled tensors)."""
    NTB = T // TB
    x = np.asarray(x, np.float32).reshape(T, D)
    ve = np.asarray(ve, np.float32).reshape(T, NH * DH)
    lam = np.asarray(lambdas, np.float32)
    qkv_w = np.asarray(qkv_w, np.float32)
    proj_w = np.asarray(proj_w, np.float32)

    # x tiled transposed: X[b, p, c, u] = x[b*512+u, c*128+p]
    xt = np.ascontiguousarray(
        x.reshape(NTB, TB, D // 128, 128).transpose(0, 3, 2, 1)).astype(bf16)

    quarter = DH // 4
    ang = (1.0 / 1024.0) ** np.linspace(0.0, 1.0, quarter, dtype=np.float32)
    theta = np.arange(T, dtype=np.float32)[:, None] * ang[None, :]   # [T, 32]
    # tiled: C[p, g, f] = cos[g*128+p, f]
    cos_t = np.ascontiguousarray(
        np.cos(theta).astype(np.float32).reshape(T // 128, 128, 32).transpose(1, 0, 2))
    sin_t = np.ascontiguousarray(
        np.sin(theta).astype(np.float32).reshape(T // 128, 128, 32).transpose(1, 0, 2))

    s_l = np.arange(128)[:, None]
    t_l = np.arange(TB)[None, :]
    mask = np.stack([(t_l >= s_l + 128 * j) for j in range(4)], axis=1).astype(bf16)  # [128,4,TB]

    in_maps = []
    for c in range(NCORES):
        sl = slice(c * DLOC, (c + 1) * DLOC)
        # columns: q0 q1 k0 k1 (heads split at 128) then v
        wqkv = np.concatenate(
            [qkv_w[0, sl].T, qkv_w[1, sl].T, lam[0] * qkv_w[2, sl].T], axis=1)  # [D, 768]
        wq_t = np.ascontiguousarray(
            wqkv.reshape(D // 128, 128, 3 * DLOC).transpose(1, 0, 2)).astype(bf16)
        wp = proj_w[:, sl].T  # [DLOC, D]
        wp_t = np.ascontiguousarray(
            wp.reshape(HPC, 128, D).transpose(1, 0, 2)).astype(bf16)
        ve_c = (lam[1] * ve[:, sl])
        ve_t = np.ascontiguousarray(
            ve_c.reshape(NTB, 4, 128, DLOC).transpose(0, 2, 1, 3)).astype(bf16)
        in_maps.append({
            "xt": xt,
            "wqkv": wq_t,
            "wproj": wp_t,
            "ve": ve_t,
            "cos": cos_t, "sin": sin_t, "mask": mask,
        })
    return in_maps


def kernel(x, ve, lambdas, qkv_w, proj_w):
    B, T, _ = x.shape
    in_maps = _host_prep(x, ve, lambdas, qkv_w, proj_w, T)
    if T not in _BUILD_CACHE:
        nc = _build(T)
        nc.compile()
        _BUILD_CACHE[T] = nc
    nc = _BUILD_CACHE[T]

    from concourse.bass_utils import run_bass_kernel_spmd
    res = run_bass_kernel_spmd(nc, in_maps, core_ids=list(range(NCORES)))
    out = np.zeros((T, D), np.float32)
    for c in range(NCORES):
        out += res.results[c]["out"]
    return out.reshape(B, T, D)


# revision 18
# speedup vs baseline: 1.4063x; 1.1669x over previous
"""Causal self-attention (RMS-normed QK + partial RoPE + lambda-blended V)
for Trainium2, tensor-parallel over heads across 8 NeuronCores.

Per core: 2 heads. Host pre-tiles all inputs (x transposed to [d, t] bf16
tiles) so every DMA is 128 contiguous per-partition descriptors and the
device does no casts / transposes of x.

Device pipeline per 512-token block:
  xt tile DMA -> QKV matmuls (x-tile stationary, bf16, fp32 psum)
  -> RMS (ACT square+accum, exp(-.5 ln)) -> RoPE (DVE, q+k fused tiles)
  -> PE-transpose q,k into resident [d, t] tiles
  -> flash-style causal attention (scores^T per 128-key sub-block, ACT exp,
     diagonal mask, bf16 l accumulation, PV accumulation in psum)
  -> l partition-reduced via ones-matmul (fp32 psum), reciprocal,
     PE outer-product broadcast of 1/l, fold into o
  -> out projection with both heads accumulated in one psum bank,
     ACT evacuation -> partial out [T, D] fp32.
Host: shards weights per core, sums the 8 partial outputs.
"""
import sys
sys.path.insert(0, "/opt/trn_rl_repo")

import math
import numpy as np
import ml_dtypes

import concourse.bass as bass
import concourse.tile as tile
from concourse import bacc, mybir
from concourse.masks import make_identity

bf16 = ml_dtypes.bfloat16
F32 = mybir.dt.float32
BF = mybir.dt.bfloat16
AF = mybir.ActivationFunctionType
ALU = mybir.AluOpType

D = 2048          # model dim
NH = 16           # total heads
DH = 128          # head dim
NCORES = 8
HPC = NH // NCORES          # heads per core = 2
DLOC = HPC * DH             # local hdim = 256
EPS = 1e-6
TB = 512                    # t-block size
SQRT_DH = math.sqrt(DH)

_BUILD_CACHE = {}


def _patch_act_tables():
    """Force the act-table-load pass to serve every activation from
    `natural_log_exp_and_others` (it contains Exp, Ln, Square, Copy,
    Identity — everything this kernel uses). The default chooser picks
    exp_and_others for Exp and natural_log for Ln, which alternate every
    sub-block and cost ~1.3us per reload. Indices into act_info.json are
    preserved, so the emitted set id stays valid at runtime."""
    import concourse.bacc as bacc_mod
    import concourse.hw_specs as hw_specs
    if getattr(bacc_mod, "_act_tables_patched", False):
        return
    orig = hw_specs.get_activation_tables

    def patched(arch):
        t = orig(arch)
        keep = "natural_log_exp_and_others"
        return {name: (fns if name == keep else set()) for name, fns in t.items()}

    bacc_mod.get_activation_tables = patched
    bacc_mod._act_tables_patched = True


def _build(T):
    """Build the per-core Bass program (same program on all cores)."""
    NTB = T // TB
    NC128 = T // 128
    _patch_act_tables()
    nc = bacc.Bacc("TRN2", target_bir_lowering=False)

    xt_in = nc.dram_tensor("xt", [NTB, 128, D // 128, TB], BF, kind="ExternalInput")
    ve_in = nc.dram_tensor("ve", [NTB, 128, 4, DLOC], BF, kind="ExternalInput")
    wq_in = nc.dram_tensor("wqkv", [128, D // 128, 3 * DLOC], BF, kind="ExternalInput")
    wp_in = nc.dram_tensor("wproj", [128, HPC, D], BF, kind="ExternalInput")
    cos_in = nc.dram_tensor("cos", [128, NC128, 32], F32, kind="ExternalInput")
    sin_in = nc.dram_tensor("sin", [128, NC128, 32], F32, kind="ExternalInput")
    mask_in = nc.dram_tensor("mask", [128, 128], BF, kind="ExternalInput")
    out_d = nc.dram_tensor("out", [T, D], F32, kind="ExternalOutput")

    with tile.TileContext(nc) as tc:
        with (
            tc.tile_pool(name="const", bufs=1) as const,
            tc.tile_pool(name="res", bufs=1) as res,
            tc.tile_pool(name="xt", bufs=2) as xtp,
            tc.tile_pool(name="vep", bufs=2) as vep,
            tc.tile_pool(name="work", bufs=2) as work,
            tc.tile_pool(name="att", bufs=3) as att,
            tc.tile_pool(name="lac", bufs=2) as lac,
            tc.tile_pool(name="lrw", bufs=2) as lrw,
            tc.tile_pool(name="lbc", bufs=2) as lbcp,
            tc.tile_pool(name="ob", bufs=2) as obp,
            tc.tile_pool(name="prj", bufs=2) as prj,
            tc.tile_pool(name="psA", bufs=2, space="PSUM") as psA,
            tc.tile_pool(name="psB", bufs=1, space="PSUM") as psB,
            tc.tile_pool(name="psC", bufs=1, space="PSUM") as psC,
            tc.tile_pool(name="psD", bufs=2, space="PSUM") as psD,
        ):
            # ---------------- constants ----------------
            wq_sb = const.tile([128, D // 128, 3 * DLOC], BF, tag="wq")
            nc.sync.dma_start(wq_sb[:], wq_in[:])
            wp_sb = const.tile([128, HPC, D], BF, tag="wp")
            nc.sync.dma_start(wp_sb[:], wp_in[:])
            cos_sb = const.tile([128, NC128, 32], F32, tag="cos")
            nc.sync.dma_start(cos_sb[:], cos_in[:])
            sin_sb = const.tile([128, NC128, 32], F32, tag="sin")
            nc.sync.dma_start(sin_sb[:], sin_in[:])
            mask_sb = const.tile([128, 128], BF, tag="mask")
            nc.sync.dma_start(mask_sb[:], mask_in[:])
            ident = const.tile([128, 128], BF, tag="ident")
            make_identity(nc, ident[:])
            ones_bf = const.tile([128, 1], BF, tag="onesb")
            nc.vector.memset(ones_bf[:], 1.0)
            ones1_f = const.tile([1, 128], F32, tag="ones1")
            nc.vector.memset(ones1_f[:], 1.0)
            lnbias = const.tile([128, 1], F32, tag="lnbias")
            nc.vector.memset(lnbias[:], float(EPS * SQRT_DH))

            # ---------------- resident per-block tensors ----------------
            # qkT[i]: [d, (q0,q1,k0,k1), t] transposed q/k
            qkT = [res.tile([128, 4, TB], BF, tag=f"qkT{i}", name=f"qkT{i}")
                   for i in range(NTB)]
            vB = [res.tile([128, 4, DLOC], BF, tag=f"v{i}", name=f"v{i}")
                  for i in range(NTB)]

            for ti in range(NTB):
                # ============ QKV stage ============
                xt = xtp.tile([128, D // 128, TB], BF, tag="xt")
                nc.sync.dma_start(xt[:], xt_in[ti])
                ve_sb = vep.tile([128, 4, DLOC], BF, tag="ve")
                nc.sync.dma_start(ve_sb[:], ve_in[ti])

                for sub in range(4):
                    tg = ti * 4 + sub
                    qkv_ps = psA.tile([128, 1024], F32, tag="big")
                    ndc = D // 128
                    for dc in range(ndc):
                        lhsT = xt[:, dc, sub * 128:(sub + 1) * 128]
                        st, sp = dc == 0, dc == ndc - 1
                        # q0|q1|k0|k1 share psum bank 0; v is bank 1
                        nc.tensor.matmul(qkv_ps[:, 0:512], lhsT, wq_sb[:, dc, 0:512], start=st, stop=sp)
                        nc.tensor.matmul(qkv_ps[:, 512:768], lhsT, wq_sb[:, dc, 512:768], start=st, stop=sp)
                    # v blend: vB = v_psum + lam1*ve (lam0 folded in weights)
                    nc.vector.tensor_add(vB[ti][:, sub, :], qkv_ps[:, 512:768], ve_sb[:, sub, :])
                    # evacuate raw q|k once (frees psum bank fast), then rms
                    # stats via DVE bn_stats (mean-square = var + mean^2)
                    qk_raw = work.tile([128, 4, 128], BF, tag="qkraw")
                    nc.vector.tensor_copy(qk_raw[:], qkv_ps[:, 0:512])
                    st6 = work.tile([128, 4, 6], F32, tag="st6")
                    st2 = work.tile([128, 4, 2], F32, tag="st2")
                    for i in range(4):
                        nc.vector.bn_stats(st6[:, i, :], qk_raw[:, i, :])
                        nc.vector.bn_aggr(st2[:, i, :], st6[:, i, :])
                    ms = work.tile([128, 4], F32, tag="ms")
                    nc.vector.tensor_tensor(ms[:], st2[:, :, 0], st2[:, :, 0], ALU.mult)
                    nc.vector.tensor_add(ms[:], ms[:], st2[:, :, 1])
                    # rstd' = (sqrt(DH)*(ms+eps))^-1/2; folds the 1/sqrt(DH)
                    # score scale (split as DH^-0.25 into q and k each)
                    rstd = work.tile([128, 4], F32, tag="rstd")
                    nc.scalar.activation(rstd[:], ms[:], AF.Ln,
                                         scale=float(SQRT_DH), bias=lnbias[:])
                    nc.scalar.activation(rstd[:], rstd[:], AF.Exp, scale=-0.5)
                    # normalize + cast, q0 q1 k0 k1 into one fused tile
                    qkn = work.tile([128, 4, DH], BF, tag="qkn")
                    for i in range(4):
                        nc.vector.tensor_scalar_mul(qkn[:, i, :], qk_raw[:, i, :], rstd[:, i:i + 1])
                    # rope (first 32 freq pairs only; rest identity), q&k fused
                    cosb = cos_sb[:, tg, :][:, None, :].broadcast_to([128, 4, 32])
                    sinb = sin_sb[:, tg, :][:, None, :].broadcast_to([128, 4, 32])
                    x1 = qkn[:, :, 0:32]
                    x2 = qkn[:, :, 64:96]
                    r1 = work.tile([128, 4, 32], BF, tag="r1")
                    r2 = work.tile([128, 4, 32], BF, tag="r2")
                    r3 = work.tile([128, 4, 32], BF, tag="r3")
                    r4 = work.tile([128, 4, 32], BF, tag="r4")
                    nc.vector.tensor_mul(r1[:], x1, cosb)
                    nc.vector.tensor_mul(r2[:], x2, sinb)
                    nc.vector.tensor_mul(r3[:], x1, sinb)
                    nc.vector.tensor_mul(r4[:], x2, cosb)
                    nc.vector.tensor_add(x1, r1[:], r2[:])
                    nc.vector.tensor_sub(x2, r4[:], r3[:])
                    # transpose q,k into resident [d, t] layout; batch the
                    # 4 results into one psum tile -> single copy out
                    tp4 = psC.tile([128, 4, 128], BF, tag="tp4")
                    for i in range(4):
                        nc.tensor.transpose(tp4[:, i, :], qkn[:, i, :], ident[:])
                    nc.any.tensor_copy(qkT[ti][:, :, sub * 128:(sub + 1) * 128], tp4[:])

                # ============ attention stage ============
                oB = obp.tile([128, HPC, TB], BF, tag="o")
                ns = (ti + 1) * 4
                for h in range(HPC):
                    l_acc = lac.tile([128, 1024], BF, tag="lacc")
                    nc.vector.memset(l_acc[:], 0.0)
                    o_ps = psB.tile([128, TB], F32, tag="o")
                    for sj2 in range(0, ns, 2):
                        # diagonal sub-blocks: queries t < sb_*128 cannot see
                        # this key block, so compute only t in [off, TB)
                        diag_pair = sj2 >= ti * 4
                        offs = [(sj2 + k2) % 4 * 128 if diag_pair else 0 for k2 in range(2)]
                        sc_ps = psA.tile([128, 1024], F32, tag="big")
                        for k2 in range(2):
                            sj = sj2 + k2
                            blk, sb_ = sj // 4, sj % 4
                            off = offs[k2]
                            nc.tensor.matmul(
                                sc_ps[:, k2 * 512 + off:(k2 + 1) * 512],
                                qkT[blk][:, 2 + h, sb_ * 128:(sb_ + 1) * 128],
                                qkT[ti][:, h, off:TB], start=True, stop=True)
                        probs = att.tile([128, 1024], BF, tag="probs")
                        if not diag_pair:
                            nc.scalar.activation(probs[:], sc_ps[:], AF.Exp)
                        else:
                            for k2 in range(2):
                                off = offs[k2]
                                nc.scalar.activation(
                                    probs[:, k2 * 512 + off:(k2 + 1) * 512],
                                    sc_ps[:, k2 * 512 + off:(k2 + 1) * 512], AF.Exp)
                                # triangular mask on the 128-wide diagonal
                                nc.vector.tensor_mul(
                                    probs[:, k2 * 512 + off:k2 * 512 + off + 128],
                                    probs[:, k2 * 512 + off:k2 * 512 + off + 128], mask_sb[:])
                        # l accumulation (bf16; partition-reduce later is fp32)
                        for k2 in range(2):
                            off = offs[k2]
                            nc.vector.tensor_add(
                                l_acc[:, k2 * 512 + off:(k2 + 1) * 512],
                                l_acc[:, k2 * 512 + off:(k2 + 1) * 512],
                                probs[:, k2 * 512 + off:(k2 + 1) * 512])
                        for k2 in range(2):
                            sj = sj2 + k2
                            blk, sb_ = sj // 4, sj % 4
                            off = offs[k2]
                            nc.tensor.matmul(
                                o_ps[:, off:TB], vB[blk][:, sb_, h * 128:(h + 1) * 128],
                                probs[:, k2 * 512 + off:(k2 + 1) * 512],
                                start=(sj == 0), stop=(sj == ns - 1))
                    # l[t] = sum_s l_acc[s, t] via ones-matmul (fp32 in psum)
                    lr = psD.tile([1, TB], F32, tag="pr")
                    nc.tensor.matmul(lr[:], ones_bf[:], l_acc[:, 0:512], start=True, stop=False)
                    nc.tensor.matmul(lr[:], ones_bf[:], l_acc[:, 512:1024], start=False, stop=True)
                    linv_row = lrw.tile([1, TB], F32, tag="linv")
                    nc.vector.reciprocal(linv_row[:], lr[:])
                    # broadcast 1/l across partitions via PE outer product
                    lbc_ps = psD.tile([128, TB], F32, tag="pr")
                    nc.tensor.matmul(lbc_ps[:], ones1_f[:], linv_row[:], start=True, stop=True)
                    lbc = lbcp.tile([128, TB], BF, tag="lbc")
                    nc.scalar.copy(lbc[:], lbc_ps[:])
                    # fold 1/l into o while evacuating psum
                    nc.vector.tensor_mul(oB[:, h, :], o_ps[:], lbc[:])

                # ============ projection stage ============
                t0 = ti * TB
                for sub in range(4):
                    out_sb = prj.tile([128, D], F32, tag="outsb")
                    for dn in range(D // 512):
                        pr = psD.tile([128, 512], F32, tag="pr")
                        nc.tensor.matmul(pr[:], oB[:, 0, sub * 128:(sub + 1) * 128],
                                         wp_sb[:, 0, dn * 512:(dn + 1) * 512], start=True, stop=False)
                        nc.tensor.matmul(pr[:], oB[:, 1, sub * 128:(sub + 1) * 128],
                                         wp_sb[:, 1, dn * 512:(dn + 1) * 512], start=False, stop=True)
                        nc.scalar.copy(out_sb[:, dn * 512:(dn + 1) * 512], pr[:])
                    nc.sync.dma_start(out_d[t0 + sub * 128: t0 + (sub + 1) * 128, :], out_sb[:])
    return nc


def _host_prep(x, ve, lambdas, qkv_w, proj_w, T):
    """Build the 8 per-core input maps (sharding + pre-tiled tensors)."""
    NTB = T // TB
    x = np.asarray(x, np.float32).reshape(T, D)
    ve = np.asarray(ve, np.float32).reshape(T, NH * DH)
    lam = np.asarray(lambdas, np.float32)
    qkv_w = np.asarray(qkv_w, np.float32)
    proj_w = np.asarray(proj_w, np.float32)

    # x tiled transposed: X[b, p, c, u] = x[b*512+u, c*128+p]
    xt = np.ascontiguousarray(
        x.reshape(NTB, TB, D // 128, 128).transpose(0, 3, 2, 1)).astype(bf16)

    quarter = DH // 4
    ang = (1.0 / 1024.0) ** np.linspace(0.0, 1.0, quarter, dtype=np.float32)
    theta = np.arange(T, dtype=np.float32)[:, None] * ang[None, :]   # [T, 32]
    # tiled: C[p, g, f] = cos[g*128+p, f]
    cos_t = np.ascontiguousarray(
        np.cos(theta).astype(np.float32).reshape(T // 128, 128, 32).transpose(1, 0, 2))
    sin_t = np.ascontiguousarray(
        np.sin(theta).astype(np.float32).reshape(T // 128, 128, 32).transpose(1, 0, 2))

    s_l = np.arange(128)[:, None]
    t_l = np.arange(128)[None, :]
    mask = (t_l >= s_l).astype(bf16)  # [128,128] triangular

    in_maps = []
    for c in range(NCORES):
        sl = slice(c * DLOC, (c + 1) * DLOC)
        # columns: q0 q1 k0 k1 (heads split at 128) then v
        wqkv = np.concatenate(
            [qkv_w[0, sl].T, qkv_w[1, sl].T, lam[0] * qkv_w[2, sl].T], axis=1)  # [D, 768]
        wq_t = np.ascontiguousarray(
            wqkv.reshape(D // 128, 128, 3 * DLOC).transpose(1, 0, 2)).astype(bf16)
        wp = proj_w[:, sl].T  # [DLOC, D]
        wp_t = np.ascontiguousarray(
            wp.reshape(HPC, 128, D).transpose(1, 0, 2)).astype(bf16)
        ve_c = (lam[1] * ve[:, sl])
        ve_t = np.ascontiguousarray(
            ve_c.reshape(NTB, 4, 128, DLOC).transpose(0, 2, 1, 3)).astype(bf16)
        in_maps.append({
            "xt": xt,
            "wqkv": wq_t,
            "wproj": wp_t,
            "ve": ve_t,
            "cos": cos_t, "sin": sin_t, "mask": mask,
        })
    return in_maps


def kernel(x, ve, lambdas, qkv_w, proj_w):
    B, T, _ = x.shape
    in_maps = _host_prep(x, ve, lambdas, qkv_w, proj_w, T)
    if T not in _BUILD_CACHE:
        nc = _build(T)
        nc.compile()
        _BUILD_CACHE[T] = nc
    nc = _BUILD_CACHE[T]

    from concourse.bass_utils import run_bass_kernel_spmd
    res = run_bass_kernel_spmd(nc, in_maps, core_ids=list(range(NCORES)))
    out = np.zeros((T, D), np.float32)
    for c in range(NCORES):
        out += res.results[c]["out"]
    return out.reshape(B, T, D)


# revision 21
# speedup vs baseline: 1.6138x; 1.1476x over previous
"""Causal self-attention (RMS-normed QK + partial RoPE + lambda-blended V)
for Trainium2, tensor-parallel over heads across 8 NeuronCores.

Per core: 2 heads. Host pre-tiles all inputs (x transposed to [d, t] bf16
tiles) so every DMA is 128 contiguous per-partition descriptors and the
device does no casts / transposes of x.

Device pipeline per 512-token block:
  xt tile DMA -> QKV matmuls (x-tile stationary, bf16, fp32 psum)
  -> RMS (ACT square+accum, exp(-.5 ln)) -> RoPE (DVE, q+k fused tiles)
  -> PE-transpose q,k into resident [d, t] tiles
  -> flash-style causal attention (scores^T per 128-key sub-block, ACT exp,
     diagonal mask, bf16 l accumulation, PV accumulation in psum)
  -> l partition-reduced via ones-matmul (fp32 psum), reciprocal,
     PE outer-product broadcast of 1/l, fold into o
  -> out projection with both heads accumulated in one psum bank,
     ACT evacuation -> partial out [T, D] fp32.
Host: shards weights per core, sums the 8 partial outputs.
"""
import sys
sys.path.insert(0, "/opt/trn_rl_repo")

import math
import numpy as np
import ml_dtypes

import concourse.bass as bass
import concourse.tile as tile
from concourse import bacc, mybir
from concourse.masks import make_identity

bf16 = ml_dtypes.bfloat16
F32 = mybir.dt.float32
BF = mybir.dt.bfloat16
AF = mybir.ActivationFunctionType
ALU = mybir.AluOpType

D = 2048          # model dim
NH = 16           # total heads
DH = 128          # head dim
NCORES = 8
HPC = NH // NCORES          # heads per core = 2
DLOC = HPC * DH             # local hdim = 256
EPS = 1e-6
TB = 512                    # t-block size
SQRT_DH = math.sqrt(DH)

_BUILD_CACHE = {}


def _patch_act_tables():
    """Force the act-table-load pass to serve every activation from
    `natural_log_exp_and_others` (it contains Exp, Ln, Square, Copy,
    Identity — everything this kernel uses). The default chooser picks
    exp_and_others for Exp and natural_log for Ln, which alternate every
    sub-block and cost ~1.3us per reload. Indices into act_info.json are
    preserved, so the emitted set id stays valid at runtime."""
    import concourse.bacc as bacc_mod
    import concourse.hw_specs as hw_specs
    if getattr(bacc_mod, "_act_tables_patched", False):
        return
    orig = hw_specs.get_activation_tables

    def patched(arch):
        t = orig(arch)
        keep = "natural_log_exp_and_others"
        return {name: (fns if name == keep else set()) for name, fns in t.items()}

    bacc_mod.get_activation_tables = patched
    bacc_mod._act_tables_patched = True


def _build(T):
    """Build the per-core Bass program (same program on all cores)."""
    NTB = T // TB
    NC128 = T // 128
    _patch_act_tables()
    nc = bacc.Bacc("TRN2", target_bir_lowering=False)

    xt_in = nc.dram_tensor("xt", [NTB, 128, D // 128, TB], BF, kind="ExternalInput")
    ve_in = nc.dram_tensor("ve", [NTB, 128, 4, DLOC], BF, kind="ExternalInput")
    wq_in = nc.dram_tensor("wqkv", [128, D // 128, 3 * DLOC], BF, kind="ExternalInput")
    wp_in = nc.dram_tensor("wproj", [128, HPC, D], BF, kind="ExternalInput")
    cos_in = nc.dram_tensor("cos", [128, NC128, 32], F32, kind="ExternalInput")
    sin_in = nc.dram_tensor("sin", [128, NC128, 32], F32, kind="ExternalInput")
    mask_in = nc.dram_tensor("mask", [128, 128], BF, kind="ExternalInput")
    out_d = nc.dram_tensor("out", [T, D], F32, kind="ExternalOutput")

    with tile.TileContext(nc) as tc:
        with (
            tc.tile_pool(name="const", bufs=1) as const,
            tc.tile_pool(name="res", bufs=1) as res,
            tc.tile_pool(name="xt", bufs=2) as xtp,
            tc.tile_pool(name="vep", bufs=2) as vep,
            tc.tile_pool(name="work", bufs=2) as work,
            tc.tile_pool(name="att", bufs=4) as att,
            tc.tile_pool(name="lac", bufs=2) as lac,
            tc.tile_pool(name="lrw", bufs=2) as lrw,
            tc.tile_pool(name="lbc", bufs=2) as lbcp,
            tc.tile_pool(name="ob", bufs=2) as obp,
            tc.tile_pool(name="prj", bufs=2) as prj,
            tc.tile_pool(name="psA", bufs=2, space="PSUM") as psA,
            tc.tile_pool(name="psB", bufs=1, space="PSUM") as psB,
            tc.tile_pool(name="psD", bufs=2, space="PSUM") as psD,
        ):
            # ---------------- constants ----------------
            wq_sb = const.tile([128, D // 128, 3 * DLOC], BF, tag="wq")
            nc.sync.dma_start(wq_sb[:], wq_in[:])
            wp_sb = const.tile([128, HPC, D], BF, tag="wp")
            nc.sync.dma_start(wp_sb[:], wp_in[:])
            cos_sb = const.tile([128, NC128, 32], F32, tag="cos")
            nc.sync.dma_start(cos_sb[:], cos_in[:])
            sin_sb = const.tile([128, NC128, 32], F32, tag="sin")
            nc.sync.dma_start(sin_sb[:], sin_in[:])
            mask_sb = const.tile([128, 128], BF, tag="mask")
            nc.sync.dma_start(mask_sb[:], mask_in[:])
            ident = const.tile([128, 128], BF, tag="ident")
            make_identity(nc, ident[:])
            ones_bf = const.tile([128, 1], BF, tag="onesb")
            nc.vector.memset(ones_bf[:], 1.0)
            ones1_f = const.tile([1, 128], F32, tag="ones1")
            nc.vector.memset(ones1_f[:], 1.0)
            lnbias = const.tile([128, 1], F32, tag="lnbias")
            nc.vector.memset(lnbias[:], float(EPS * SQRT_DH))

            # ---------------- resident per-block tensors ----------------
            # qkT[i]: [d, (q0,q1,k0,k1), t] transposed q/k
            qkT = [res.tile([128, 4, TB], BF, tag=f"qkT{i}", name=f"qkT{i}")
                   for i in range(NTB)]
            vB = [res.tile([128, 4, DLOC], BF, tag=f"v{i}", name=f"v{i}")
                  for i in range(NTB)]

            ndc = D // 128

            def emit_qkv_sub(ti, sub, xt, ve_sb):
                tg = ti * 4 + sub
                qkv_ps = psA.tile([128, 1024], F32, tag="big")
                for dc in range(ndc):
                    lhsT = xt[:, dc, sub * 128:(sub + 1) * 128]
                    st, sp = dc == 0, dc == ndc - 1
                    # q0|q1|k0|k1 share psum bank 0; v is bank 1
                    nc.tensor.matmul(qkv_ps[:, 0:512], lhsT, wq_sb[:, dc, 0:512], start=st, stop=sp)
                    nc.tensor.matmul(qkv_ps[:, 512:768], lhsT, wq_sb[:, dc, 512:768], start=st, stop=sp)
                # v blend: vB = v_psum + lam1*ve (lam0 folded in weights)
                nc.vector.tensor_add(vB[ti][:, sub, :], qkv_ps[:, 512:768], ve_sb[:, sub, :])
                # evacuate raw q|k once (frees psum bank fast), then rms
                # stats via DVE bn_stats (mean-square = var + mean^2)
                qk_raw = work.tile([128, 4, 128], BF, tag="qkraw")
                nc.vector.tensor_copy(qk_raw[:], qkv_ps[:, 0:512])
                st6 = work.tile([128, 4, 6], F32, tag="st6")
                st2 = work.tile([128, 4, 2], F32, tag="st2")
                for i in range(4):
                    nc.vector.bn_stats(st6[:, i, :], qk_raw[:, i, :])
                    nc.vector.bn_aggr(st2[:, i, :], st6[:, i, :])
                ms = work.tile([128, 4], F32, tag="ms")
                nc.vector.tensor_tensor(ms[:], st2[:, :, 0], st2[:, :, 0], ALU.mult)
                nc.vector.tensor_add(ms[:], ms[:], st2[:, :, 1])
                # rstd' = (sqrt(DH)*(ms+eps))^-1/2; folds the 1/sqrt(DH)
                # score scale (split as DH^-0.25 into q and k each)
                rstd = work.tile([128, 4], F32, tag="rstd")
                nc.scalar.activation(rstd[:], ms[:], AF.Ln,
                                     scale=float(SQRT_DH), bias=lnbias[:])
                nc.scalar.activation(rstd[:], rstd[:], AF.Exp, scale=-0.5)
                # normalize + cast, q0 q1 k0 k1 into one fused tile
                qkn = work.tile([128, 4, DH], BF, tag="qkn")
                for i in range(4):
                    nc.vector.tensor_scalar_mul(qkn[:, i, :], qk_raw[:, i, :], rstd[:, i:i + 1])
                # rope (first 32 freq pairs only; rest identity), q&k fused
                cosb = cos_sb[:, tg, :][:, None, :].broadcast_to([128, 4, 32])
                sinb = sin_sb[:, tg, :][:, None, :].broadcast_to([128, 4, 32])
                x1 = qkn[:, :, 0:32]
                x2 = qkn[:, :, 64:96]
                r1 = work.tile([128, 4, 32], BF, tag="r1")
                r2 = work.tile([128, 4, 32], BF, tag="r2")
                r3 = work.tile([128, 4, 32], BF, tag="r3")
                r4 = work.tile([128, 4, 32], BF, tag="r4")
                nc.vector.tensor_mul(r1[:], x1, cosb)
                nc.vector.tensor_mul(r2[:], x2, sinb)
                nc.vector.tensor_mul(r3[:], x1, sinb)
                nc.vector.tensor_mul(r4[:], x2, cosb)
                nc.vector.tensor_add(x1, r1[:], r2[:])
                nc.vector.tensor_sub(x2, r4[:], r3[:])
                # transpose q,k into resident [d, t] layout; batch the
                # 4 results into one psum tile -> single copy out
                tp4 = psD.tile([128, 4, 128], BF, tag="pr")
                for i in range(4):
                    nc.tensor.transpose(tp4[:, i, :], qkn[:, i, :], ident[:])
                nc.any.tensor_copy(qkT[ti][:, :, sub * 128:(sub + 1) * 128], tp4[:])

            def emit_lbc(state):
                # broadcast 1/l across partitions (PE outer product) and fold
                # into o while evacuating psum; emitted one block late so the
                # reciprocal latency hides under QKV matmuls
                for h in range(HPC):
                    lbc_ps = psD.tile([128, TB], F32, tag="pr")
                    nc.tensor.matmul(lbc_ps[:], ones1_f[:], state["linv"][h][:], start=True, stop=True)
                    lbc = lbcp.tile([128, TB], BF, tag="lbc")
                    nc.scalar.copy(lbc[:], lbc_ps[:])
                    nc.vector.tensor_mul(state["oB"][:, h, :], state["o_ps"][:, h, :], lbc[:])

            def emit_proj(state, ti_prev):
                # out projection, both heads accumulated in one psum bank
                t0 = ti_prev * TB
                oB = state["oB"]
                for sub in range(4):
                    out_sb = prj.tile([128, D], F32, tag="outsb")
                    for dn in range(D // 512):
                        pr = psD.tile([128, 512], F32, tag="pr")
                        nc.tensor.matmul(pr[:], oB[:, 0, sub * 128:(sub + 1) * 128],
                                         wp_sb[:, 0, dn * 512:(dn + 1) * 512], start=True, stop=False)
                        nc.tensor.matmul(pr[:], oB[:, 1, sub * 128:(sub + 1) * 128],
                                         wp_sb[:, 1, dn * 512:(dn + 1) * 512], start=False, stop=True)
                        nc.scalar.copy(out_sb[:, dn * 512:(dn + 1) * 512], pr[:])
                    nc.sync.dma_start(out_d[t0 + sub * 128: t0 + (sub + 1) * 128, :], out_sb[:])

            pending = None
            for ti in range(NTB):
                # ============ QKV stage (+ deferred prev-block epilogue) ====
                xt = xtp.tile([128, D // 128, TB], BF, tag="xt")
                nc.sync.dma_start(xt[:], xt_in[ti])
                ve_sb = vep.tile([128, 4, DLOC], BF, tag="ve")
                nc.sync.dma_start(ve_sb[:], ve_in[ti])

                emit_qkv_sub(ti, 0, xt, ve_sb)
                if pending is not None:
                    emit_lbc(pending)
                for sub in range(1, 4):
                    emit_qkv_sub(ti, sub, xt, ve_sb)
                if pending is not None:
                    emit_proj(pending, ti - 1)
                    pending = None

                # ============ attention stage (heads interleaved) ============
                oB = obp.tile([128, HPC, TB], BF, tag="o")
                o_ps = psB.tile([128, HPC, TB], F32, tag="o")
                l_acc = [lac.tile([128, TB], BF, tag=f"lacc{h}", name=f"lacc{h}") for h in range(HPC)]
                for h in range(HPC):
                    nc.gpsimd.memset(l_acc[h][:], 0.0)
                ns = (ti + 1) * 4
                for sj2 in range(0, ns, 2):
                    # diagonal sub-blocks: queries t < sb_*128 cannot see
                    # this key block, so compute only t in [off, TB)
                    diag_pair = sj2 >= ti * 4
                    offs = [(sj2 + k2) % 4 * 128 if diag_pair else 0 for k2 in range(2)]
                    sc = []
                    for h in range(HPC):
                        sc_ps = psA.tile([128, 1024], F32, tag="big")
                        sc.append(sc_ps)
                        for k2 in range(2):
                            sj = sj2 + k2
                            blk, sb_ = sj // 4, sj % 4
                            off = offs[k2]
                            nc.tensor.matmul(
                                sc_ps[:, k2 * 512 + off:(k2 + 1) * 512],
                                qkT[blk][:, 2 + h, sb_ * 128:(sb_ + 1) * 128],
                                qkT[ti][:, h, off:TB], start=True, stop=True)
                    prb = []
                    for h in range(HPC):
                        probs = att.tile([128, 1024], BF, tag="probs")
                        prb.append(probs)
                        if not diag_pair:
                            nc.scalar.activation(probs[:], sc[h][:], AF.Exp)
                        else:
                            for k2 in range(2):
                                off = offs[k2]
                                nc.scalar.activation(
                                    probs[:, k2 * 512 + off:(k2 + 1) * 512],
                                    sc[h][:, k2 * 512 + off:(k2 + 1) * 512], AF.Exp)
                                # triangular mask on the 128-wide diagonal
                                nc.vector.tensor_mul(
                                    probs[:, k2 * 512 + off:k2 * 512 + off + 128],
                                    probs[:, k2 * 512 + off:k2 * 512 + off + 128], mask_sb[:])
                    # l accumulation (bf16; partition-reduce later is fp32)
                    for h in range(HPC):
                        for k2 in range(2):
                            off = offs[k2]
                            nc.vector.tensor_add(
                                l_acc[h][:, off:TB], l_acc[h][:, off:TB],
                                prb[h][:, k2 * 512 + off:(k2 + 1) * 512])
                    for h in range(HPC):
                        for k2 in range(2):
                            sj = sj2 + k2
                            blk, sb_ = sj // 4, sj % 4
                            off = offs[k2]
                            nc.tensor.matmul(
                                o_ps[:, h, off:TB], vB[blk][:, sb_, h * 128:(h + 1) * 128],
                                prb[h][:, k2 * 512 + off:(k2 + 1) * 512],
                                start=(sj == 0), stop=(sj == ns - 1))
                # l[t] = sum_s l_acc[s, t] via ones-matmul (fp32 in psum);
                # the rest of the epilogue is deferred into the next block
                linvs = []
                for h in range(HPC):
                    lr = psD.tile([1, TB], F32, tag="pr")
                    nc.tensor.matmul(lr[:], ones_bf[:], l_acc[h][:], start=True, stop=True)
                    linv_row = lrw.tile([1, TB], F32, tag=f"linv{h}")
                    nc.vector.reciprocal_approx_fast(linv_row[:], lr[:])
                    linvs.append(linv_row)
                pending = {"oB": oB, "o_ps": o_ps, "linv": linvs}

            emit_lbc(pending)
            emit_proj(pending, NTB - 1)
    return nc


def _host_prep(x, ve, lambdas, qkv_w, proj_w, T):
    """Build the 8 per-core input maps (sharding + pre-tiled tensors)."""
    NTB = T // TB
    x = np.asarray(x, np.float32).reshape(T, D)
    ve = np.asarray(ve, np.float32).reshape(T, NH * DH)
    lam = np.asarray(lambdas, np.float32)
    qkv_w = np.asarray(qkv_w, np.float32)
    proj_w = np.asarray(proj_w, np.float32)

    # x tiled transposed: X[b, p, c, u] = x[b*512+u, c*128+p]
    xt = np.ascontiguousarray(
        x.reshape(NTB, TB, D // 128, 128).transpose(0, 3, 2, 1)).astype(bf16)

    quarter = DH // 4
    ang = (1.0 / 1024.0) ** np.linspace(0.0, 1.0, quarter, dtype=np.float32)
    theta = np.arange(T, dtype=np.float32)[:, None] * ang[None, :]   # [T, 32]
    # tiled: C[p, g, f] = cos[g*128+p, f]
    cos_t = np.ascontiguousarray(
        np.cos(theta).astype(np.float32).reshape(T // 128, 128, 32).transpose(1, 0, 2))
    sin_t = np.ascontiguousarray(
        np.sin(theta).astype(np.float32).reshape(T // 128, 128, 32).transpose(1, 0, 2))

    s_l = np.arange(128)[:, None]
    t_l = np.arange(128)[None, :]
    mask = (t_l >= s_l).astype(bf16)  # [128,128] triangular

    in_maps = []
    for c in range(NCORES):
        sl = slice(c * DLOC, (c + 1) * DLOC)
        # columns: q0 q1 k0 k1 (heads split at 128) then v
        wqkv = np.concatenate(
            [qkv_w[0, sl].T, qkv_w[1, sl].T, lam[0] * qkv_w[2, sl].T], axis=1)  # [D, 768]
        wq_t = np.ascontiguousarray(
            wqkv.reshape(D // 128, 128, 3 * DLOC).transpose(1, 0, 2)).astype(bf16)
        wp = proj_w[:, sl].T  # [DLOC, D]
        wp_t = np.ascontiguousarray(
            wp.reshape(HPC, 128, D).transpose(1, 0, 2)).astype(bf16)
        ve_c = (lam[1] * ve[:, sl])
        ve_t = np.ascontiguousarray(
            ve_c.reshape(NTB, 4, 128, DLOC).transpose(0, 2, 1, 3)).astype(bf16)
        in_maps.append({
            "xt": xt,
            "wqkv": wq_t,
            "wproj": wp_t,
            "ve": ve_t,
            "cos": cos_t, "sin": sin_t, "mask": mask,
        })
    return in_maps


def kernel(x, ve, lambdas, qkv_w, proj_w):
    B, T, _ = x.shape
    in_maps = _host_prep(x, ve, lambdas, qkv_w, proj_w, T)
    if T not in _BUILD_CACHE:
        nc = _build(T)
        nc.compile()
        _BUILD_CACHE[T] = nc
    nc = _BUILD_CACHE[T]

    from concourse.bass_utils import run_bass_kernel_spmd
    res = run_bass_kernel_spmd(nc, in_maps, core_ids=list(range(NCORES)))
    out = np.zeros((T, D), np.float32)
    for c in range(NCORES):
        out += res.results[c]["out"]
    return out.reshape(B, T, D)
